# revision 9
# baseline (speedup 1.0000x reference)
"""DPFM loss kernel for 8 Trainium2 NeuronCores.

Loss = frobenius(C12, C_gt) + weighted_bce(ov12, gt12) + weighted_bce(ov21, gt21)
       + 0.1 * nce_softmax(feat1, feat2, map21)

Sharding: the 4096x4096 NCE similarity/CE is sharded over query rows
(512 queries per core). Each core gathers its 512 q rows and all 4096 k
rows from the full feat tables with device-side indirect DMAs, streamed
in 128-row chunks so normalize/transpose/matmul/sqrt pipeline behind
the gather stream. The key order is permuted per core (host-side index
shuffle, order-invariant for the softmax row-sum) so the core's own
matched diagonal keys arrive first. Per-query sumexp is computed on
device via a fused exp+row-sum on the scalar engine; BCE / frobenius
terms are per-partition partial sums. The host only sums partials and
applies the final log (the unshard step).
"""

import numpy as np

N_CORES = 8
N = 100000
D = 128
P = 4096
PC = P // N_CORES          # 512 queries per core
NB = PC // 128             # 4 q blocks of 128 rows
NK = P // 128              # 32 key chunks of 128 rows
NG = 8                     # key chunk groups (4 chunks each) -> 8 groups of 4? no: NK//GS
GS = 4                     # chunks per group
NS = N // N_CORES          # 12500 BCE elements per core
BCE_P, BCE_F = 125, 100    # 12500 = 125 x 100
T = 0.07
W_NCE = 0.1

_cache = {}


def _build():
    from concourse import bass, bacc, mybir, tile
    from concourse.masks import make_identity

    f32, bf16, i32 = mybir.dt.float32, mybir.dt.bfloat16, mybir.dt.int32
    AF = mybir.ActivationFunctionType
    OP = mybir.AluOpType
    AX = mybir.AxisListType

    nc = bacc.Bacc(None, target_bir_lowering=False, debug=True, num_devices=N_CORES)

    f1 = nc.dram_tensor("f1", [N, D], f32, kind="ExternalInput")
    f2 = nc.dram_tensor("f2", [N, D], f32, kind="ExternalInput")
    qidx = nc.dram_tensor("qidx", [128, NB], i32, kind="ExternalInput")
    kidx = nc.dram_tensor("kidx", [128, NK], i32, kind="ExternalInput")
    ov = nc.dram_tensor("ov", [BCE_P, 2 * BCE_F], f32, kind="ExternalInput")
    gt = nc.dram_tensor("gt", [BCE_P, 2 * BCE_F], i32, kind="ExternalInput")
    c12 = nc.dram_tensor("c12", [100, 100], f32, kind="ExternalInput")
    cgt = nc.dram_tensor("cgt", [100, 100], f32, kind="ExternalInput")

    out_sums = nc.dram_tensor("out_sums", [128, 2 * NB], f32, kind="ExternalOutput")
    out_dii = nc.dram_tensor("out_dii", [128, NB], f32, kind="ExternalOutput")
    out_misc = nc.dram_tensor("out_misc", [128, 12], f32, kind="ExternalOutput")

    n_groups = NK // GS  # 8 groups of 4 chunks (512 keys per group)

    with tile.TileContext(nc) as tc:
        with tc.tile_pool(name="const", bufs=1) as cpool, \
             tc.tile_pool(name="persist", bufs=1) as gpool, \
             tc.tile_pool(name="scratch", bufs=3) as spool, \
             tc.tile_pool(name="gscr", bufs=3) as gsp, \
             tc.tile_pool(name="expscr", bufs=2) as epool, \
             tc.tile_pool(name="tpsum", bufs=2, space="PSUM") as tpp, \
             tc.tile_pool(name="spsum", bufs=5, space="PSUM") as spp:

            # ---- consts + small input loads (issued first) ----
            qidx_t = cpool.tile([128, NB], i32)
            kidx_t = cpool.tile([128, NK], i32)
            nc.sync.dma_start(qidx_t[:], qidx[:])
            nc.sync.dma_start(kidx_t[:], kidx[:])
            ident = cpool.tile([128, 128], f32)
            make_identity(nc, ident[:])
            two = cpool.tile([128, 1], f32)
            nc.vector.memset(two[:], 2.0)
            ov_t = cpool.tile([BCE_P, 2 * BCE_F], f32)
            gt_t = cpool.tile([BCE_P, 2 * BCE_F], i32)
            nc.sync.dma_start(ov_t[:], ov[:])
            nc.sync.dma_start(gt_t[:], gt[:])
            c12_t = cpool.tile([100, 100], f32)
            cgt_t = cpool.tile([100, 100], f32)
            nc.sync.dma_start(c12_t[:], c12[:])
            nc.sync.dma_start(cgt_t[:], cgt[:])

            # ---- BCE log inputs early: the two Ln ops run during warm-up idle ----
            gtf = gpool.tile([BCE_P, 2 * BCE_F], f32)
            nc.vector.tensor_copy(gtf[:], gt_t[:])
            pcl = gpool.tile([BCE_P, 2 * BCE_F], f32)
            nc.vector.tensor_scalar_max(pcl[:], ov_t[:], 1e-38)
            logp = gpool.tile([BCE_P, 2 * BCE_F], f32)
            nc.scalar.activation(out=logp[:], in_=pcl[:], func=AF.Ln)
            logq = gpool.tile([BCE_P, 2 * BCE_F], f32)
            nc.scalar.activation(out=logq[:], in_=ov_t[:], func=AF.Ln,
                                 scale=-1.0, bias=1.0)

            # ---- q gathers first (qT needed by every matmul), then k stream ----
            gq = gpool.tile([128, NB, D], f32)
            for j in range(NB):
                nc.gpsimd.indirect_dma_start(
                    out=gq[:, j, :], out_offset=None, in_=f1[:],
                    in_offset=bass.IndirectOffsetOnAxis(ap=qidx_t[:, j:j + 1], axis=0))

            gk_all = gpool.tile([128, NK, D], f32)
            for m in range(NK):
                nc.gpsimd.indirect_dma_start(
                    out=gk_all[:, m, :], out_offset=None, in_=f2[:],
                    in_offset=bass.IndirectOffsetOnAxis(
                        ap=kidx_t[:, m:m + 1], axis=0))

            # ---- q side: norms -> normalize -> transpose -> qT ----
            norms_q = gpool.tile([128, NB], f32)
            for j in range(NB):
                sq = spool.tile([128, D], f32, tag="sq")
                nc.vector.tensor_mul(sq[:], gq[:, j, :], gq[:, j, :])
                nc.vector.tensor_reduce(out=norms_q[:, j:j + 1], in_=sq[:],
                                        axis=AX.X, op=OP.add)
            nstd_q = gpool.tile([128, NB], f32)
            nc.scalar.activation(out=nstd_q[:], in_=norms_q[:], func=AF.Sqrt)
            inv_q = gpool.tile([128, NB], f32)
            nc.vector.reciprocal(inv_q[:], nstd_q[:])
            qn = gpool.tile([128, NB, D], f32)
            qT = gpool.tile([128, PC], bf16)
            for j in range(NB):
                nc.vector.tensor_scalar(out=qn[:, j, :], in0=gq[:, j, :],
                                        scalar1=inv_q[:, j:j + 1], scalar2=None,
                                        op0=OP.mult)
                trp = tpp.tile([128, 128], f32, tag="trp")
                nc.tensor.transpose(out=trp[:], in_=qn[:, j, :], identity=ident[:])
                nc.vector.tensor_copy(qT[:, j * 128:(j + 1) * 128], trp[:])

            # ---- k stream: per group normalize (in place) + transpose,
            #      then matmul+sqrt; bulk exp runs mid-stream (split 6/2) ----
            kT = gpool.tile([128, P], bf16)
            d_all = gpool.tile([128, NB, P], bf16)
            sums = gpool.tile([128, 2 * NB], f32)   # cols j: groups 0-5, NB+j: 6-7
            G_SPLIT = 6

            def k_group(g):
                norms_k = gsp.tile([128, GS], f32, tag="nk")
                for i in range(GS):
                    m = g * GS + i
                    sq = spool.tile([128, D], f32, tag="sq")
                    nc.vector.tensor_mul(sq[:], gk_all[:, m, :], gk_all[:, m, :])
                    nc.vector.tensor_reduce(out=norms_k[:, i:i + 1], in_=sq[:],
                                            axis=AX.X, op=OP.add)
                nstd_k = gsp.tile([128, GS], f32, tag="nsk")
                nc.scalar.activation(out=nstd_k[:], in_=norms_k[:], func=AF.Sqrt)
                inv_k = gsp.tile([128, GS], f32, tag="ivk")
                nc.vector.reciprocal(inv_k[:], nstd_k[:])
                for i in range(GS):
                    m = g * GS + i
                    nc.vector.tensor_scalar(out=gk_all[:, m, :], in0=gk_all[:, m, :],
                                            scalar1=inv_k[:, i:i + 1], scalar2=None,
                                            op0=OP.mult)
                    trp = tpp.tile([128, 128], f32, tag="trp")
                    nc.tensor.transpose(out=trp[:], in_=gk_all[:, m, :],
                                        identity=ident[:])
                    nc.vector.tensor_copy(kT[:, m * 128:(m + 1) * 128], trp[:])
                # S = qT_j.T @ kT_group ; d = sqrt(2 - 2 S)
                for j in range(NB):
                    S = spp.tile([128, GS * 128], f32, tag="S")
                    nc.tensor.matmul(
                        S[:], lhsT=qT[:, j * 128:(j + 1) * 128],
                        rhs=kT[:, g * GS * 128:(g + 1) * GS * 128],
                        start=True, stop=True)
                    nc.scalar.activation(
                        out=d_all[:, j, g * GS * 128:(g + 1) * GS * 128], in_=S[:],
                        func=AF.Sqrt, scale=-2.0, bias=two[:, :1])

                if g == 0:
                    # diagonal: d_ii from matched (q_i, k_i) pairs (chunks 0-3)
                    sii = gpool.tile([128, NB], f32)
                    for j in range(NB):
                        dp = spool.tile([128, D], f32, tag="sq")
                        nc.vector.tensor_mul(dp[:], qn[:, j, :],
                                             gk_all[:, j, :])
                        nc.vector.tensor_reduce(out=sii[:, j:j + 1], in_=dp[:],
                                                axis=AX.X, op=OP.add)
                    dii = gpool.tile([128, NB], f32)
                    nc.scalar.activation(out=dii[:], in_=sii[:], func=AF.Sqrt,
                                         scale=-2.0, bias=two[:, :1])
                    nc.sync.dma_start(out_dii[:], dii[:])

            for g in range(G_SPLIT):
                k_group(g)
            # bulk exp over groups 0..G_SPLIT-1 (ACT slack inside the stream)
            c_split = G_SPLIT * GS * 128
            for j in range(NB):
                w = epool.tile([128, c_split], bf16, tag="w")
                nc.scalar.activation(out=w[:], in_=d_all[:, j, :c_split],
                                     func=AF.Exp, scale=-1.0 / T,
                                     accum_out=sums[:, j:j + 1])
            for g in range(G_SPLIT, n_groups):
                k_group(g)
            # tail exp over the remaining groups
            for j in range(NB):
                w = epool.tile([128, P - c_split], bf16, tag="w2")
                nc.scalar.activation(out=w[:], in_=d_all[:, j, c_split:],
                                     func=AF.Exp, scale=-1.0 / T,
                                     accum_out=sums[:, NB + j:NB + j + 1])
            nc.sync.dma_start(out_sums[:], sums[:])

            # ---- BCE partial sums (cheap DVE work, fills stream slack) ----
            misc = gpool.tile([128, 12], f32)
            nc.vector.memset(misc[:], 0.0)
            nc.vector.tensor_scalar_max(logp[:], logp[:], -100.0)
            c1g = gpool.tile([BCE_P, 2 * BCE_F], f32)
            nc.vector.tensor_mul(c1g[:], logp[:], gtf[:])
            nc.vector.tensor_scalar_max(logq[:], logq[:], -100.0)
            c0g = gpool.tile([BCE_P, 2 * BCE_F], f32)
            nc.vector.tensor_mul(c0g[:], logq[:], gtf[:])
            for h in range(2):
                cs = slice(h * BCE_F, (h + 1) * BCE_F)
                base = 4 * h
                nc.vector.tensor_reduce(out=misc[:BCE_P, base:base + 1],
                                        in_=gtf[:, cs], axis=AX.X, op=OP.add)
                nc.vector.tensor_reduce(out=misc[:BCE_P, base + 1:base + 2],
                                        in_=c1g[:, cs], axis=AX.X, op=OP.add)
                nc.vector.tensor_reduce(out=misc[:BCE_P, base + 2:base + 3],
                                        in_=logq[:, cs], axis=AX.X, op=OP.add)
                nc.vector.tensor_reduce(out=misc[:BCE_P, base + 3:base + 4],
                                        in_=c0g[:, cs], axis=AX.X, op=OP.add)

            # ---- frobenius partial sums ----
            cd = spool.tile([100, 100], f32, tag="fmap")
            nc.vector.tensor_sub(cd[:], c12_t[:], cgt_t[:])
            csq = spool.tile([100, 100], f32, tag="fmap")
            nc.vector.tensor_mul(csq[:], cd[:], cd[:])
            nc.vector.tensor_reduce(out=misc[:100, 8:9], in_=csq[:],
                                    axis=AX.X, op=OP.add)
            nc.sync.dma_start(out_misc[:], misc[:])

    nc.finalize()
    return nc


def _prepare_in_maps(C12, C_gt, map21, feat1, feat2, overlap_score12,
                     overlap_score21, gt_partiality_mask12, gt_partiality_mask21):
    f1 = np.ascontiguousarray(feat1, dtype=np.float32)
    f2 = np.ascontiguousarray(feat2, dtype=np.float32)
    c12 = np.ascontiguousarray(np.asarray(C12).reshape(100, 100), dtype=np.float32)
    cgt = np.ascontiguousarray(np.asarray(C_gt).reshape(100, 100), dtype=np.float32)
    m = np.asarray(map21, dtype=np.int32)
    o12 = np.asarray(overlap_score12, dtype=np.float32)
    o21 = np.asarray(overlap_score21, dtype=np.float32)
    g12 = np.asarray(gt_partiality_mask12, dtype=np.int32)
    g21 = np.asarray(gt_partiality_mask21, dtype=np.int32)

    in_maps = []
    for c in range(N_CORES):
        qs = m[c * PC:(c + 1) * PC, 0]
        # key order is irrelevant for the softmax row-sum; put this core's
        # matched diag keys (pairs c*PC..c*PC+PC-1) in the first 4 chunks
        perm = np.concatenate([
            np.arange(c * PC, (c + 1) * PC),
            np.arange(0, c * PC),
            np.arange((c + 1) * PC, P),
        ])
        ks = m[perm, 1]
        sl = slice(c * NS, (c + 1) * NS)
        in_maps.append({
            "f1": f1,
            "f2": f2,
            "qidx": np.ascontiguousarray(qs.reshape(NB, 128).T),
            "kidx": np.ascontiguousarray(ks.reshape(NK, 128).T),
            "ov": np.ascontiguousarray(np.concatenate(
                [o12[sl].reshape(BCE_P, BCE_F), o21[sl].reshape(BCE_P, BCE_F)],
                axis=1)),
            "gt": np.ascontiguousarray(np.concatenate(
                [g12[sl].reshape(BCE_P, BCE_F), g21[sl].reshape(BCE_P, BCE_F)],
                axis=1)),
            "c12": c12,
            "cgt": cgt,
        })
    return in_maps


last_exec_time_ns = None


def kernel(**inputs) -> np.ndarray:
    global last_exec_time_ns
    from concourse.bass_utils import run_bass_kernel_spmd

    if "nc" not in _cache:
        _cache["nc"] = _build()
    nc = _cache["nc"]

    in_maps = _prepare_in_maps(**inputs)
    res = run_bass_kernel_spmd(nc, in_maps, list(range(N_CORES)))
    last_exec_time_ns = res.exec_time_ns

    # ---- host unshard: sum partials, final log for lse ----
    nce_sum = 0.0
    S = np.zeros(9, dtype=np.float64)
    for c in range(N_CORES):
        sums2 = np.asarray(res.results[c]["out_sums"], np.float64)
        sums = sums2[:, :NB] + sums2[:, NB:]
        dii = np.asarray(res.results[c]["out_dii"], np.float64)
        nce_sum += (np.log(sums) + dii / T).sum()
        S += np.asarray(res.results[c]["out_misc"], np.float64)[:, :9].sum(axis=0)
    nce = W_NCE * nce_sum / P

    acc = 0.0
    for h in range(2):
        s_gt, s1, s_l0, s_gl0 = S[4 * h:4 * h + 4]
        w_neg = s_gt / N
        w_pos = 1.0 - w_neg
        s0 = s_l0 - s_gl0
        acc += -(w_pos * s1 + w_neg * s0) / N

    # fmap partials are identical on every core; use core 0's copy
    fmap = np.asarray(res.results[0]["out_misc"], np.float64)[:, 8].sum()

    return np.asarray(fmap + acc + nce, dtype=np.float32)


# revision 10
# speedup vs baseline: 1.0031x; 1.0031x over previous
"""DPFM loss kernel for 8 Trainium2 NeuronCores.

Loss = frobenius(C12, C_gt) + weighted_bce(ov12, gt12) + weighted_bce(ov21, gt21)
       + 0.1 * nce_softmax(feat1, feat2, map21)

Sharding: the 4096x4096 NCE similarity/CE is sharded over query rows
(512 queries per core). Each core gathers its 512 q rows and all 4096 k
rows from the full feat tables with device-side indirect DMAs, streamed
in 128-row chunks so normalize/transpose/matmul/sqrt pipeline behind
the gather stream. The key order is permuted per core (host-side index
shuffle, order-invariant for the softmax row-sum) so the core's own
matched diagonal keys arrive first. Per-query sumexp is computed on
device via a fused exp+row-sum on the scalar engine; BCE / frobenius
terms are per-partition partial sums. The host only sums partials and
applies the final log (the unshard step).
"""

import numpy as np

N_CORES = 8
N = 100000
D = 128
P = 4096
PC = P // N_CORES          # 512 queries per core
NB = PC // 128             # 4 q blocks of 128 rows
NK = P // 128              # 32 key chunks of 128 rows
NG = 8                     # key chunk groups (4 chunks each) -> 8 groups of 4? no: NK//GS
GS = 4                     # chunks per group
NS = N // N_CORES          # 12500 BCE elements per core
BCE_P, BCE_F = 125, 100    # 12500 = 125 x 100
T = 0.07
W_NCE = 0.1

_cache = {}


def _build():
    from concourse import bass, bacc, mybir, tile
    from concourse.masks import make_identity

    f32, bf16, i32 = mybir.dt.float32, mybir.dt.bfloat16, mybir.dt.int32
    AF = mybir.ActivationFunctionType
    OP = mybir.AluOpType
    AX = mybir.AxisListType

    nc = bacc.Bacc(None, target_bir_lowering=False, debug=True, num_devices=N_CORES)

    f1 = nc.dram_tensor("f1", [N, D], f32, kind="ExternalInput")
    f2 = nc.dram_tensor("f2", [N, D], f32, kind="ExternalInput")
    qidx = nc.dram_tensor("qidx", [128, NB], i32, kind="ExternalInput")
    kidx = nc.dram_tensor("kidx", [128, NK], i32, kind="ExternalInput")
    ov = nc.dram_tensor("ov", [BCE_P, 2 * BCE_F], f32, kind="ExternalInput")
    gt = nc.dram_tensor("gt", [BCE_P, 2 * BCE_F], i32, kind="ExternalInput")
    c12 = nc.dram_tensor("c12", [100, 100], f32, kind="ExternalInput")
    cgt = nc.dram_tensor("cgt", [100, 100], f32, kind="ExternalInput")

    out_sums = nc.dram_tensor("out_sums", [128, NB], f32, kind="ExternalOutput")
    out_dii = nc.dram_tensor("out_dii", [128, NB], f32, kind="ExternalOutput")
    out_misc = nc.dram_tensor("out_misc", [128, 12], f32, kind="ExternalOutput")

    n_groups = NK // GS  # 8 groups of 4 chunks (512 keys per group)

    with tile.TileContext(nc) as tc:
        with tc.tile_pool(name="const", bufs=1) as cpool, \
             tc.tile_pool(name="persist", bufs=1) as gpool, \
             tc.tile_pool(name="scratch", bufs=3) as spool, \
             tc.tile_pool(name="gscr", bufs=3) as gsp, \
             tc.tile_pool(name="expscr", bufs=2) as epool, \
             tc.tile_pool(name="tpsum", bufs=2, space="PSUM") as tpp, \
             tc.tile_pool(name="spsum", bufs=5, space="PSUM") as spp:

            # ---- consts + small input loads (issued first) ----
            qidx_t = cpool.tile([128, NB], i32)
            kidx_t = cpool.tile([128, NK], i32)
            nc.sync.dma_start(qidx_t[:], qidx[:])
            nc.sync.dma_start(kidx_t[:], kidx[:])
            ident = cpool.tile([128, 128], f32)
            make_identity(nc, ident[:])
            two = cpool.tile([128, 1], f32)
            nc.vector.memset(two[:], 2.0)
            ov_t = cpool.tile([BCE_P, 2 * BCE_F], f32)
            gt_t = cpool.tile([BCE_P, 2 * BCE_F], i32)
            nc.sync.dma_start(ov_t[:], ov[:])
            nc.sync.dma_start(gt_t[:], gt[:])
            c12_t = cpool.tile([100, 100], f32)
            cgt_t = cpool.tile([100, 100], f32)
            nc.sync.dma_start(c12_t[:], c12[:])
            nc.sync.dma_start(cgt_t[:], cgt[:])

            # ---- BCE log inputs early: the two Ln ops run during warm-up idle ----
            gtf = gpool.tile([BCE_P, 2 * BCE_F], f32)
            nc.vector.tensor_copy(gtf[:], gt_t[:])
            pcl = gpool.tile([BCE_P, 2 * BCE_F], f32)
            nc.vector.tensor_scalar_max(pcl[:], ov_t[:], 1e-38)
            logp = gpool.tile([BCE_P, 2 * BCE_F], f32)
            nc.scalar.activation(out=logp[:], in_=pcl[:], func=AF.Ln)
            logq = gpool.tile([BCE_P, 2 * BCE_F], f32)
            nc.scalar.activation(out=logq[:], in_=ov_t[:], func=AF.Ln,
                                 scale=-1.0, bias=1.0)

            # ---- q gathers first (qT needed by every matmul), then k stream ----
            gq = gpool.tile([128, NB, D], f32)
            for j in range(NB):
                nc.gpsimd.indirect_dma_start(
                    out=gq[:, j, :], out_offset=None, in_=f1[:],
                    in_offset=bass.IndirectOffsetOnAxis(ap=qidx_t[:, j:j + 1], axis=0))

            gk_tiles = []
            for g in range(n_groups):
                gk = gsp.tile([128, GS, D], f32, tag="gk")
                gk_tiles.append(gk)
                for i in range(GS):
                    m = g * GS + i
                    nc.gpsimd.indirect_dma_start(
                        out=gk[:, i, :], out_offset=None, in_=f2[:],
                        in_offset=bass.IndirectOffsetOnAxis(
                            ap=kidx_t[:, m:m + 1], axis=0))

            # ---- q side: norms -> normalize -> transpose -> qT ----
            norms_q = gpool.tile([128, NB], f32)
            for j in range(NB):
                sq = spool.tile([128, D], f32, tag="sq")
                nc.vector.tensor_mul(sq[:], gq[:, j, :], gq[:, j, :])
                nc.vector.tensor_reduce(out=norms_q[:, j:j + 1], in_=sq[:],
                                        axis=AX.X, op=OP.add)
            nstd_q = gpool.tile([128, NB], f32)
            nc.scalar.activation(out=nstd_q[:], in_=norms_q[:], func=AF.Sqrt)
            inv_q = gpool.tile([128, NB], f32)
            nc.vector.reciprocal(inv_q[:], nstd_q[:])
            qn = gpool.tile([128, NB, D], f32)
            qT = gpool.tile([128, PC], bf16)
            for j in range(NB):
                nc.vector.tensor_scalar(out=qn[:, j, :], in0=gq[:, j, :],
                                        scalar1=inv_q[:, j:j + 1], scalar2=None,
                                        op0=OP.mult)
                trp = tpp.tile([128, 128], f32, tag="trp")
                nc.tensor.transpose(out=trp[:], in_=qn[:, j, :], identity=ident[:])
                nc.vector.tensor_copy(qT[:, j * 128:(j + 1) * 128], trp[:])

            # ---- k stream: per group normalize+transpose, then matmul+sqrt ----
            kT = gpool.tile([128, P], bf16)
            d_all = gpool.tile([128, NB, P], bf16)
            kn0 = gpool.tile([128, GS, D], f32)   # group 0 = this core's diag keys
            for g in range(n_groups):
                gk = gk_tiles[g]
                norms_k = gsp.tile([128, GS], f32, tag="nk")
                for i in range(GS):
                    sq = spool.tile([128, D], f32, tag="sq")
                    nc.vector.tensor_mul(sq[:], gk[:, i, :], gk[:, i, :])
                    nc.vector.tensor_reduce(out=norms_k[:, i:i + 1], in_=sq[:],
                                            axis=AX.X, op=OP.add)
                nstd_k = gsp.tile([128, GS], f32, tag="nsk")
                nc.scalar.activation(out=nstd_k[:], in_=norms_k[:], func=AF.Sqrt)
                inv_k = gsp.tile([128, GS], f32, tag="ivk")
                nc.vector.reciprocal(inv_k[:], nstd_k[:])
                for i in range(GS):
                    if g == 0:
                        knt = kn0[:, i, :]
                    else:
                        kns = gsp.tile([128, D], f32, tag="kn")
                        knt = kns[:]
                    nc.vector.tensor_scalar(out=knt, in0=gk[:, i, :],
                                            scalar1=inv_k[:, i:i + 1], scalar2=None,
                                            op0=OP.mult)
                    trp = tpp.tile([128, 128], f32, tag="trp")
                    nc.tensor.transpose(out=trp[:], in_=knt, identity=ident[:])
                    m = g * GS + i
                    nc.vector.tensor_copy(kT[:, m * 128:(m + 1) * 128], trp[:])
                # S = qT_j.T @ kT_group ; d = sqrt(2 - 2 S)
                for j in range(NB):
                    S = spp.tile([128, GS * 128], f32, tag="S")
                    nc.tensor.matmul(
                        S[:], lhsT=qT[:, j * 128:(j + 1) * 128],
                        rhs=kT[:, g * GS * 128:(g + 1) * GS * 128],
                        start=True, stop=True)
                    nc.scalar.activation(
                        out=d_all[:, j, g * GS * 128:(g + 1) * GS * 128], in_=S[:],
                        func=AF.Sqrt, scale=-2.0, bias=two[:, :1])

                if g == 0:
                    # diagonal: d_ii from matched (q_i, k_i) pairs (chunks 0-3)
                    sii = gpool.tile([128, NB], f32)
                    for j in range(NB):
                        dp = spool.tile([128, D], f32, tag="sq")
                        nc.vector.tensor_mul(dp[:], qn[:, j, :], kn0[:, j, :])
                        nc.vector.tensor_reduce(out=sii[:, j:j + 1], in_=dp[:],
                                                axis=AX.X, op=OP.add)
                    dii = gpool.tile([128, NB], f32)
                    nc.scalar.activation(out=dii[:], in_=sii[:], func=AF.Sqrt,
                                         scale=-2.0, bias=two[:, :1])
                    nc.sync.dma_start(out_dii[:], dii[:])

            # ---- exp pass with fused row-sum ----
            sums = gpool.tile([128, NB], f32)
            for j in range(NB):
                w = epool.tile([128, P], bf16, tag="w")
                nc.scalar.activation(out=w[:], in_=d_all[:, j, :], func=AF.Exp,
                                     scale=-1.0 / T, accum_out=sums[:, j:j + 1])
            nc.sync.dma_start(out_sums[:], sums[:])

            # ---- BCE partial sums (cheap DVE work, fills stream slack) ----
            misc = gpool.tile([128, 12], f32)
            nc.vector.memset(misc[:], 0.0)
            nc.vector.tensor_scalar_max(logp[:], logp[:], -100.0)
            c1g = gpool.tile([BCE_P, 2 * BCE_F], f32)
            nc.vector.tensor_mul(c1g[:], logp[:], gtf[:])
            nc.vector.tensor_scalar_max(logq[:], logq[:], -100.0)
            c0g = gpool.tile([BCE_P, 2 * BCE_F], f32)
            nc.vector.tensor_mul(c0g[:], logq[:], gtf[:])
            for h in range(2):
                cs = slice(h * BCE_F, (h + 1) * BCE_F)
                base = 4 * h
                nc.vector.tensor_reduce(out=misc[:BCE_P, base:base + 1],
                                        in_=gtf[:, cs], axis=AX.X, op=OP.add)
                nc.vector.tensor_reduce(out=misc[:BCE_P, base + 1:base + 2],
                                        in_=c1g[:, cs], axis=AX.X, op=OP.add)
                nc.vector.tensor_reduce(out=misc[:BCE_P, base + 2:base + 3],
                                        in_=logq[:, cs], axis=AX.X, op=OP.add)
                nc.vector.tensor_reduce(out=misc[:BCE_P, base + 3:base + 4],
                                        in_=c0g[:, cs], axis=AX.X, op=OP.add)

            # ---- frobenius partial sums ----
            cd = spool.tile([100, 100], f32, tag="fmap")
            nc.vector.tensor_sub(cd[:], c12_t[:], cgt_t[:])
            csq = spool.tile([100, 100], f32, tag="fmap")
            nc.vector.tensor_mul(csq[:], cd[:], cd[:])
            nc.vector.tensor_reduce(out=misc[:100, 8:9], in_=csq[:],
                                    axis=AX.X, op=OP.add)
            nc.sync.dma_start(out_misc[:], misc[:])

    nc.finalize()
    return nc


def _prepare_in_maps(C12, C_gt, map21, feat1, feat2, overlap_score12,
                     overlap_score21, gt_partiality_mask12, gt_partiality_mask21):
    f1 = np.ascontiguousarray(feat1, dtype=np.float32)
    f2 = np.ascontiguousarray(feat2, dtype=np.float32)
    c12 = np.ascontiguousarray(np.asarray(C12).reshape(100, 100), dtype=np.float32)
    cgt = np.ascontiguousarray(np.asarray(C_gt).reshape(100, 100), dtype=np.float32)
    m = np.asarray(map21, dtype=np.int32)
    o12 = np.asarray(overlap_score12, dtype=np.float32)
    o21 = np.asarray(overlap_score21, dtype=np.float32)
    g12 = np.asarray(gt_partiality_mask12, dtype=np.int32)
    g21 = np.asarray(gt_partiality_mask21, dtype=np.int32)

    in_maps = []
    for c in range(N_CORES):
        qs = m[c * PC:(c + 1) * PC, 0]
        # key order is irrelevant for the softmax row-sum; put this core's
        # matched diag keys (pairs c*PC..c*PC+PC-1) in the first 4 chunks
        perm = np.concatenate([
            np.arange(c * PC, (c + 1) * PC),
            np.arange(0, c * PC),
            np.arange((c + 1) * PC, P),
        ])
        ks = m[perm, 1]
        sl = slice(c * NS, (c + 1) * NS)
        in_maps.append({
            "f1": f1,
            "f2": f2,
            "qidx": np.ascontiguousarray(qs.reshape(NB, 128).T),
            "kidx": np.ascontiguousarray(ks.reshape(NK, 128).T),
            "ov": np.ascontiguousarray(np.concatenate(
                [o12[sl].reshape(BCE_P, BCE_F), o21[sl].reshape(BCE_P, BCE_F)],
                axis=1)),
            "gt": np.ascontiguousarray(np.concatenate(
                [g12[sl].reshape(BCE_P, BCE_F), g21[sl].reshape(BCE_P, BCE_F)],
                axis=1)),
            "c12": c12,
            "cgt": cgt,
        })
    return in_maps


last_exec_time_ns = None


def kernel(**inputs) -> np.ndarray:
    global last_exec_time_ns
    from concourse.bass_utils import run_bass_kernel_spmd

    if "nc" not in _cache:
        _cache["nc"] = _build()
    nc = _cache["nc"]

    in_maps = _prepare_in_maps(**inputs)
    res = run_bass_kernel_spmd(nc, in_maps, list(range(N_CORES)))
    last_exec_time_ns = res.exec_time_ns

    # ---- host unshard: sum partials, final log for lse ----
    nce_sum = 0.0
    S = np.zeros(9, dtype=np.float64)
    for c in range(N_CORES):
        sums = np.asarray(res.results[c]["out_sums"], np.float64)
        dii = np.asarray(res.results[c]["out_dii"], np.float64)
        nce_sum += (np.log(sums) + dii / T).sum()
        S += np.asarray(res.results[c]["out_misc"], np.float64)[:, :9].sum(axis=0)
    nce = W_NCE * nce_sum / P

    acc = 0.0
    for h in range(2):
        s_gt, s1, s_l0, s_gl0 = S[4 * h:4 * h + 4]
        w_neg = s_gt / N
        w_pos = 1.0 - w_neg
        s0 = s_l0 - s_gl0
        acc += -(w_pos * s1 + w_neg * s0) / N

    # fmap partials are identical on every core; use core 0's copy
    fmap = np.asarray(res.results[0]["out_misc"], np.float64)[:, 8].sum()

    return np.asarray(fmap + acc + nce, dtype=np.float32)


# revision 11
# speedup vs baseline: 1.0747x; 1.0713x over previous
"""DPFM loss kernel for 8 Trainium2 NeuronCores.

Loss = frobenius(C12, C_gt) + weighted_bce(ov12, gt12) + weighted_bce(ov21, gt21)
       + 0.1 * nce_softmax(feat1, feat2, map21)

Sharding: the 4096x4096 NCE similarity/CE is sharded over query rows
(512 queries per core). Each core gathers its 512 q rows and all 4096 k
rows from the full feat tables with device-side indirect DMAs, streamed
in 128-row chunks so normalize/transpose/matmul/sqrt pipeline behind
the gather stream. The key order is permuted per core (host-side index
shuffle, order-invariant for the softmax row-sum) so the core's own
matched diagonal keys arrive first. Per-query sumexp is computed on
device via a fused exp+row-sum on the scalar engine; BCE / frobenius
terms are per-partition partial sums. The host only sums partials and
applies the final log (the unshard step).
"""

import numpy as np

N_CORES = 8
N = 100000
D = 128
P = 4096
PC = P // N_CORES          # 512 queries per core
NB = PC // 128             # 4 q blocks of 128 rows
NK = P // 128              # 32 key chunks of 128 rows
NG = 8                     # key chunk groups (4 chunks each) -> 8 groups of 4? no: NK//GS
GS = 4                     # chunks per group
NS = N // N_CORES          # 12500 BCE elements per core
BCE_P, BCE_F = 125, 100    # 12500 = 125 x 100
T = 0.07
W_NCE = 0.1

_cache = {}


def _build():
    from concourse import bass, bacc, mybir, tile
    from concourse.masks import make_identity

    f32, bf16, i32 = mybir.dt.float32, mybir.dt.bfloat16, mybir.dt.int32
    AF = mybir.ActivationFunctionType
    OP = mybir.AluOpType
    AX = mybir.AxisListType

    nc = bacc.Bacc(None, target_bir_lowering=False, debug=True, num_devices=N_CORES)

    f1 = nc.dram_tensor("f1", [N, D], f32, kind="ExternalInput")
    f2 = nc.dram_tensor("f2", [N, D], f32, kind="ExternalInput")
    qidx = nc.dram_tensor("qidx", [128, NB], i32, kind="ExternalInput")
    kidx = nc.dram_tensor("kidx", [128, NK], i32, kind="ExternalInput")
    ov = nc.dram_tensor("ov", [BCE_P, 2 * BCE_F], f32, kind="ExternalInput")
    gt = nc.dram_tensor("gt", [BCE_P, 2 * BCE_F], i32, kind="ExternalInput")
    c12 = nc.dram_tensor("c12", [100, 100], f32, kind="ExternalInput")
    cgt = nc.dram_tensor("cgt", [100, 100], f32, kind="ExternalInput")

    out_sums = nc.dram_tensor("out_sums", [128, NB], f32, kind="ExternalOutput")
    out_dii = nc.dram_tensor("out_dii", [128, NB], f32, kind="ExternalOutput")
    out_misc = nc.dram_tensor("out_misc", [128, 12], f32, kind="ExternalOutput")

    n_groups = NK // GS  # 8 groups of 4 chunks (512 keys per group)

    with tile.TileContext(nc) as tc:
        with tc.tile_pool(name="const", bufs=1) as cpool, \
             tc.tile_pool(name="persist", bufs=1) as gpool, \
             tc.tile_pool(name="scratch", bufs=3) as spool, \
             tc.tile_pool(name="gscr", bufs=3) as gsp, \
             tc.tile_pool(name="expscr", bufs=2) as epool, \
             tc.tile_pool(name="tpsum", bufs=2, space="PSUM") as tpp, \
             tc.tile_pool(name="spsum", bufs=3, space="PSUM") as spp:

            # ---- consts + small input loads (issued first) ----
            qidx_t = cpool.tile([128, NB], i32)
            kidx_t = cpool.tile([128, NK], i32)
            nc.sync.dma_start(qidx_t[:], qidx[:])
            nc.sync.dma_start(kidx_t[:], kidx[:])
            ident = cpool.tile([128, 128], f32)
            make_identity(nc, ident[:])
            two = cpool.tile([128, 1], f32)
            nc.vector.memset(two[:], 2.0)
            ov_t = cpool.tile([BCE_P, 2 * BCE_F], f32)
            gt_t = cpool.tile([BCE_P, 2 * BCE_F], i32)
            nc.sync.dma_start(ov_t[:], ov[:])
            nc.sync.dma_start(gt_t[:], gt[:])
            c12_t = cpool.tile([100, 100], f32)
            cgt_t = cpool.tile([100, 100], f32)
            nc.sync.dma_start(c12_t[:], c12[:])
            nc.sync.dma_start(cgt_t[:], cgt[:])

            # ---- BCE log inputs early: the two Ln ops run during warm-up idle ----
            gtf = gpool.tile([BCE_P, 2 * BCE_F], f32)
            nc.vector.tensor_copy(gtf[:], gt_t[:])
            pcl = gpool.tile([BCE_P, 2 * BCE_F], f32)
            nc.vector.tensor_scalar_max(pcl[:], ov_t[:], 1e-38)
            logp = gpool.tile([BCE_P, 2 * BCE_F], f32)
            nc.scalar.activation(out=logp[:], in_=pcl[:], func=AF.Ln)
            logq = gpool.tile([BCE_P, 2 * BCE_F], f32)
            nc.scalar.activation(out=logq[:], in_=ov_t[:], func=AF.Ln,
                                 scale=-1.0, bias=1.0)

            # ---- q gathers first (qT needed by every matmul), then k stream ----
            gq = gpool.tile([128, NB, D], f32)
            for j in range(NB):
                nc.gpsimd.indirect_dma_start(
                    out=gq[:, j, :], out_offset=None, in_=f1[:],
                    in_offset=bass.IndirectOffsetOnAxis(ap=qidx_t[:, j:j + 1], axis=0))

            gk_tiles = []
            for g in range(n_groups):
                gk = gsp.tile([128, GS, D], f32, tag="gk")
                gk_tiles.append(gk)
                for i in range(GS):
                    m = g * GS + i
                    nc.gpsimd.indirect_dma_start(
                        out=gk[:, i, :], out_offset=None, in_=f2[:],
                        in_offset=bass.IndirectOffsetOnAxis(
                            ap=kidx_t[:, m:m + 1], axis=0))

            # ---- q side: norms -> normalize -> transpose -> qT ----
            norms_q = gpool.tile([128, NB], f32)
            for j in range(NB):
                sq = spool.tile([128, D], f32, tag="sq")
                nc.vector.tensor_mul(sq[:], gq[:, j, :], gq[:, j, :])
                nc.vector.tensor_reduce(out=norms_q[:, j:j + 1], in_=sq[:],
                                        axis=AX.X, op=OP.add)
            nstd_q = gpool.tile([128, NB], f32)
            nc.scalar.activation(out=nstd_q[:], in_=norms_q[:], func=AF.Sqrt)
            inv_q = gpool.tile([128, NB], f32)
            nc.vector.reciprocal(inv_q[:], nstd_q[:])
            qn = gpool.tile([128, NB, D], f32)
            qT = gpool.tile([128, PC], bf16)
            for j in range(NB):
                nc.vector.tensor_scalar(out=qn[:, j, :], in0=gq[:, j, :],
                                        scalar1=inv_q[:, j:j + 1], scalar2=None,
                                        op0=OP.mult)
                trp = tpp.tile([128, 128], f32, tag="trp")
                nc.tensor.transpose(out=trp[:], in_=qn[:, j, :], identity=ident[:])
                nc.vector.tensor_copy(qT[:, j * 128:(j + 1) * 128], trp[:])

            # ---- k stream: per group normalize+transpose, then matmul+sqrt ----
            kT = gpool.tile([128, P], bf16)
            d_all = gpool.tile([128, NB, P], bf16)
            kn0 = gpool.tile([128, GS, D], f32)   # group 0 = this core's diag keys
            for g in range(n_groups):
                gk = gk_tiles[g]
                norms_k = gsp.tile([128, GS], f32, tag="nk")
                for i in range(GS):
                    sq = spool.tile([128, D], f32, tag="sq")
                    nc.vector.tensor_mul(sq[:], gk[:, i, :], gk[:, i, :])
                    nc.vector.tensor_reduce(out=norms_k[:, i:i + 1], in_=sq[:],
                                            axis=AX.X, op=OP.add)
                nstd_k = gsp.tile([128, GS], f32, tag="nsk")
                nc.scalar.activation(out=nstd_k[:], in_=norms_k[:], func=AF.Sqrt)
                inv_k = gsp.tile([128, GS], f32, tag="ivk")
                nc.vector.reciprocal(inv_k[:], nstd_k[:])
                for i in range(GS):
                    if g == 0:
                        knt = kn0[:, i, :]
                    else:
                        kns = gsp.tile([128, D], f32, tag="kn")
                        knt = kns[:]
                    nc.vector.tensor_scalar(out=knt, in0=gk[:, i, :],
                                            scalar1=inv_k[:, i:i + 1], scalar2=None,
                                            op0=OP.mult)
                    trp = tpp.tile([128, 128], f32, tag="trp")
                    nc.tensor.transpose(out=trp[:], in_=knt, identity=ident[:])
                    m = g * GS + i
                    nc.vector.tensor_copy(kT[:, m * 128:(m + 1) * 128], trp[:])
                # S = qT_j.T @ kT_group ; d = sqrt(2 - 2 S)
                for j in range(NB):
                    S = spp.tile([128, GS * 128], f32, tag="S")
                    nc.tensor.matmul(
                        S[:], lhsT=qT[:, j * 128:(j + 1) * 128],
                        rhs=kT[:, g * GS * 128:(g + 1) * GS * 128],
                        start=True, stop=True)
                    nc.scalar.activation(
                        out=d_all[:, j, g * GS * 128:(g + 1) * GS * 128], in_=S[:],
                        func=AF.Sqrt, scale=-2.0, bias=two[:, :1])

                if g == 0:
                    # diagonal: d_ii from matched (q_i, k_i) pairs (chunks 0-3)
                    sii = gpool.tile([128, NB], f32)
                    for j in range(NB):
                        dp = spool.tile([128, D], f32, tag="sq")
                        nc.vector.tensor_mul(dp[:], qn[:, j, :], kn0[:, j, :])
                        nc.vector.tensor_reduce(out=sii[:, j:j + 1], in_=dp[:],
                                                axis=AX.X, op=OP.add)
                    dii = gpool.tile([128, NB], f32)
                    nc.scalar.activation(out=dii[:], in_=sii[:], func=AF.Sqrt,
                                         scale=-2.0, bias=two[:, :1])
                    nc.sync.dma_start(out_dii[:], dii[:])

            # ---- exp pass with fused row-sum ----
            sums = gpool.tile([128, NB], f32)
            for j in range(NB):
                w = epool.tile([128, P], bf16, tag="w")
                nc.scalar.activation(out=w[:], in_=d_all[:, j, :], func=AF.Exp,
                                     scale=-1.0 / T, accum_out=sums[:, j:j + 1])
            nc.sync.dma_start(out_sums[:], sums[:])

            # ---- BCE partial sums (cheap DVE work, fills stream slack) ----
            misc = gpool.tile([128, 12], f32)
            nc.vector.memset(misc[:], 0.0)
            nc.vector.tensor_scalar_max(logp[:], logp[:], -100.0)
            c1g = gpool.tile([BCE_P, 2 * BCE_F], f32)
            nc.vector.tensor_mul(c1g[:], logp[:], gtf[:])
            nc.vector.tensor_scalar_max(logq[:], logq[:], -100.0)
            c0g = gpool.tile([BCE_P, 2 * BCE_F], f32)
            nc.vector.tensor_mul(c0g[:], logq[:], gtf[:])
            for h in range(2):
                cs = slice(h * BCE_F, (h + 1) * BCE_F)
                base = 4 * h
                nc.vector.tensor_reduce(out=misc[:BCE_P, base:base + 1],
                                        in_=gtf[:, cs], axis=AX.X, op=OP.add)
                nc.vector.tensor_reduce(out=misc[:BCE_P, base + 1:base + 2],
                                        in_=c1g[:, cs], axis=AX.X, op=OP.add)
                nc.vector.tensor_reduce(out=misc[:BCE_P, base + 2:base + 3],
                                        in_=logq[:, cs], axis=AX.X, op=OP.add)
                nc.vector.tensor_reduce(out=misc[:BCE_P, base + 3:base + 4],
                                        in_=c0g[:, cs], axis=AX.X, op=OP.add)

            # ---- frobenius partial sums ----
            cd = spool.tile([100, 100], f32, tag="fmap")
            nc.vector.tensor_sub(cd[:], c12_t[:], cgt_t[:])
            csq = spool.tile([100, 100], f32, tag="fmap")
            nc.vector.tensor_mul(csq[:], cd[:], cd[:])
            nc.vector.tensor_reduce(out=misc[:100, 8:9], in_=csq[:],
                                    axis=AX.X, op=OP.add)
            nc.sync.dma_start(out_misc[:], misc[:])

    nc.finalize()
    return nc


def _prepare_in_maps(C12, C_gt, map21, feat1, feat2, overlap_score12,
                     overlap_score21, gt_partiality_mask12, gt_partiality_mask21):
    f1 = np.ascontiguousarray(feat1, dtype=np.float32)
    f2 = np.ascontiguousarray(feat2, dtype=np.float32)
    c12 = np.ascontiguousarray(np.asarray(C12).reshape(100, 100), dtype=np.float32)
    cgt = np.ascontiguousarray(np.asarray(C_gt).reshape(100, 100), dtype=np.float32)
    m = np.asarray(map21, dtype=np.int32)
    o12 = np.asarray(overlap_score12, dtype=np.float32)
    o21 = np.asarray(overlap_score21, dtype=np.float32)
    g12 = np.asarray(gt_partiality_mask12, dtype=np.int32)
    g21 = np.asarray(gt_partiality_mask21, dtype=np.int32)

    in_maps = []
    for c in range(N_CORES):
        qs = m[c * PC:(c + 1) * PC, 0]
        # key order is irrelevant for the softmax row-sum; put this core's
        # matched diag keys (pairs c*PC..c*PC+PC-1) in the first 4 chunks
        perm = np.concatenate([
            np.arange(c * PC, (c + 1) * PC),
            np.arange(0, c * PC),
            np.arange((c + 1) * PC, P),
        ])
        ks = m[perm, 1]
        sl = slice(c * NS, (c + 1) * NS)
        in_maps.append({
            "f1": f1,
            "f2": f2,
            "qidx": np.ascontiguousarray(qs.reshape(NB, 128).T),
            "kidx": np.ascontiguousarray(ks.reshape(NK, 128).T),
            "ov": np.ascontiguousarray(np.concatenate(
                [o12[sl].reshape(BCE_P, BCE_F), o21[sl].reshape(BCE_P, BCE_F)],
                axis=1)),
            "gt": np.ascontiguousarray(np.concatenate(
                [g12[sl].reshape(BCE_P, BCE_F), g21[sl].reshape(BCE_P, BCE_F)],
                axis=1)),
            "c12": c12,
            "cgt": cgt,
        })
    return in_maps


last_exec_time_ns = None


def kernel(**inputs) -> np.ndarray:
    global last_exec_time_ns
    from concourse.bass_utils import run_bass_kernel_spmd

    if "nc" not in _cache:
        _cache["nc"] = _build()
    nc = _cache["nc"]

    in_maps = _prepare_in_maps(**inputs)
    res = run_bass_kernel_spmd(nc, in_maps, list(range(N_CORES)))
    last_exec_time_ns = res.exec_time_ns

    # ---- host unshard: sum partials, final log for lse ----
    nce_sum = 0.0
    S = np.zeros(9, dtype=np.float64)
    for c in range(N_CORES):
        sums = np.asarray(res.results[c]["out_sums"], np.float64)
        dii = np.asarray(res.results[c]["out_dii"], np.float64)
        nce_sum += (np.log(sums) + dii / T).sum()
        S += np.asarray(res.results[c]["out_misc"], np.float64)[:, :9].sum(axis=0)
    nce = W_NCE * nce_sum / P

    acc = 0.0
    for h in range(2):
        s_gt, s1, s_l0, s_gl0 = S[4 * h:4 * h + 4]
        w_neg = s_gt / N
        w_pos = 1.0 - w_neg
        s0 = s_l0 - s_gl0
        acc += -(w_pos * s1 + w_neg * s0) / N

    # fmap partials are identical on every core; use core 0's copy
    fmap = np.asarray(res.results[0]["out_misc"], np.float64)[:, 8].sum()

    return np.asarray(fmap + acc + nce, dtype=np.float32)
